# revision 35
# baseline (speedup 1.0000x reference)
"""Multi-head causal attention (dense transformer block) on 8 TRN2 NeuronCores.

Problem: B=2, S=2048, D_MODEL=768, H=12 heads, D_HEAD=64, fp32 I/O.

Sharding: 24 (batch, head) units over 8 cores -> 3 heads x 1 batch per core.
Cores 0-3 handle batch 0 (heads 0-2, 3-5, 6-8, 9-11), cores 4-7 batch 1.
Each core computes its heads' contribution to out[b] = sum_h z_h @ W_O[h];
the host sums the 4 partials per batch and adds b_O.

The kernel is organized as a single software-pipelined stream built around
the ScalarE exp cadence (exp is the scarce resource: ~60us of ACT work):

  - scores TRANSPOSED: sT[k, q] = KT^T-slice x QT (K=64 contraction). Each
    PSUM score tile [128, 1024] holds BOTH halves of a row-group pair
    ([h0_j | h1_j], or [h2_j | h2_j+1]) so the two K=64 matmuls are
    released together and execute CONCURRENTLY in the PE array (row-group
    tiling, ~2x measured).
  - Q/K projections for later q-blocks and V projections are emitted as
    FILLER tasks interleaved into the attention rounds (one per 2 rounds),
    consuming PE slack under the ACT-bound steady state instead of
    serializing ~24us up front. They share a single 1-bank PSUM slot.
  - PV: zT_h accumulated over k-tiles in PSUM; lhsT = [V_h | ones64]
    (M=128, FWL-eligible). After the last PV the [65,512] live part is
    copied to SBUF (zsb), releasing the PSUM slot IMMEDIATELY — the
    normalize (reciprocal via a [128,12] DRAM-bounce reshape + muls) then
    runs entirely from SBUF, off the critical path.
  - PV matmuls whose PSUM slot WARs on the previous block's zsb copies are
    deferred a few tiles (q_fast skew 2 for h0, q_slow skew 4 for h1/h2)
    so they never block the strict-FIFO PE queue.
  - causal mask (memset + triangular multiply) runs on the otherwise-idle
    GpSimd engine.
  - output projection of block n is injected ~12-15 rounds into block n+1
    (by then the normalize bounce has landed); zT2's K=64 matmuls of
    adjacent q-tiles are row-group paired via a high-half copy of zT2.
"""

import numpy as np
import ml_dtypes
from contextlib import ExitStack

import concourse.bass as bass
import concourse.mybir as mybir
import concourse.tile as tile
from concourse import bacc
from concourse.bass_utils import run_bass_kernel_spmd

BF16 = mybir.dt.bfloat16
F32 = mybir.dt.float32
AF = mybir.ActivationFunctionType
NPBF16 = ml_dtypes.bfloat16

B, S, D, H, DH = 2, 2048, 768, 12, 64
N_CORES = 8
DCH = D // 128          # 6 d_model chunks
NKT = S // 128          # 16 k tiles
QB = 512                # q block width
NQB = S // QB           # 4 q blocks

TRACE_ENABLED = False
LAST_EXEC_NS = None
LAST_RESULT = None
_BUILT = None


def build_nc():
    nc = bacc.Bacc("TRN2", target_bir_lowering=False, debug=False)

    xT_d = nc.dram_tensor("xT", [D, S], BF16, kind="ExternalInput")
    wq01_d = nc.dram_tensor("wq01", [D, 128], BF16, kind="ExternalInput")
    wk01_d = nc.dram_tensor("wk01", [D, 128], BF16, kind="ExternalInput")
    wqk2_d = nc.dram_tensor("wqk2", [D, 128], BF16, kind="ExternalInput")
    wv_d = nc.dram_tensor("wv", [D, 192], BF16, kind="ExternalInput")
    wo01_d = nc.dram_tensor("wo01", [128, D], BF16, kind="ExternalInput")
    wo2_d = nc.dram_tensor("wo2", [128, D], BF16, kind="ExternalInput")
    bq01_d = nc.dram_tensor("bq01", [128, 1], F32, kind="ExternalInput")
    bk01_d = nc.dram_tensor("bk01", [128, 1], F32, kind="ExternalInput")
    bqk2_d = nc.dram_tensor("bqk2", [128, 1], F32, kind="ExternalInput")
    bv_d = nc.dram_tensor("bv", [128, 192], F32, kind="ExternalInput")
    out_d = nc.dram_tensor("out_p", [S, D], F32, kind="ExternalOutput")

    tri_np = np.triu(np.ones((128, 128), np.float32)).astype(NPBF16)
    tri_d = nc.inline_tensor(tri_np, "tri")

    with tile.TileContext(nc) as tc, ExitStack() as ctx:
        persist = ctx.enter_context(tc.tile_pool(name="persist", bufs=1))

        # ---- HAM warm-up: dummy matmuls spanning the input-DMA wait ----
        with tc.tile_pool(name="warm_ps", bufs=1, space="PSUM") as warm_pool:
            wz = persist.tile([128, 128], BF16, tag="wz")
            nc.vector.memset(wz[:], 0.0)
            wps = warm_pool.tile([128, 128], F32, tag="warm")
            for _ in range(30):
                nc.tensor.matmul(wps[:], wz[:], wz[:], start=True, stop=True)

        # ---- stage inputs in SBUF ----
        def load_w(dram, cols, tag):
            t = persist.tile([128, DCH * cols], BF16, tag=tag)
            a = dram[:, :]
            src = bass.AP(tensor=a.tensor, offset=a.offset,
                          ap=[[cols, 128], [128 * cols, DCH], [1, cols]])
            nc.sync.dma_start(t[:].rearrange("p (c f) -> p c f", c=DCH), src)
            return t

        wq01 = load_w(wq01_d, 128, "wq01")
        xt = []
        for d in range(DCH):
            t = persist.tile([128, S], BF16, tag=f"xt{d}")
            xt.append(t)
        nc.sync.dma_start(xt[0][:], xT_d[0:128, :])
        wk01 = load_w(wk01_d, 128, "wk01")
        wqk2 = load_w(wqk2_d, 128, "wqk2")
        wv = load_w(wv_d, 192, "wv")
        for d in range(1, DCH):
            nc.sync.dma_start(xt[d][:], xT_d[d * 128:(d + 1) * 128, :])

        wo01 = persist.tile([128, D], BF16, tag="wo01")
        nc.sync.dma_start(wo01[:], wo01_d[:, :])
        # wo2 duplicated on both partition halves for paired zT2 matmuls
        wo2 = persist.tile([128, D], BF16, tag="wo2")
        nc.sync.dma_start(wo2[:], wo2_d[:, :])

        def load_small(dram, shape, dt, tag):
            t = persist.tile(shape, dt, tag=tag)
            nc.sync.dma_start(t[:], dram[:, :])
            return t

        bq01 = load_small(bq01_d, [128, 1], F32, "bq01")
        bk01 = load_small(bk01_d, [128, 1], F32, "bk01")
        bqk2 = load_small(bqk2_d, [128, 1], F32, "bqk2")
        bv = load_small(bv_d, [128, 192], F32, "bv")
        tri = load_small(tri_d, [128, 128], BF16, "tri")

        # ---- persistent intermediates ----
        QT01 = persist.tile([128, S], BF16, tag="QT01")
        KT01 = persist.tile([128, S], BF16, tag="KT01")
        QKT2 = persist.tile([128, S], BF16, tag="QKT2")
        # QK2x rows 0-63 = K_h2 (low copy), rows 64-127 = Q_h2*s (high copy)
        QK2x = persist.tile([128, S], BF16, tag="QK2x")
        # V augmented with 64 ones columns per head ([V_h | ones64] x 3,
        # 384 cols per s-tile): M=128 makes LDWEIGHTS FWL-eligible and rows
        # 64-127 of the PV output hold the softmax denominator. The ones
        # blocks are constant: memset once.
        v_sb = persist.tile([128, NKT * 384], BF16, tag="v_sb")
        nc.vector.memset(
            v_sb[:].rearrange("p (t h c) -> p t h c", t=NKT, h=3)[:, :, :, 64:128],
            1.0)
        zT01 = persist.tile([128, S], BF16, tag="zT01")
        # zT2x rows 0-63 = z_h2, rows 64-127 = copy (for out-proj pairing)
        zT2x = persist.tile([128, S], BF16, tag="zT2x")

        with tc.tile_pool(name="proj_ps", bufs=1, space="PSUM") as proj_pool, \
             tc.tile_pool(name="sT_ps", bufs=2, space="PSUM") as sT_pool, \
             tc.tile_pool(name="zT_ps", bufs=3, space="PSUM") as zT_pool, \
             tc.tile_pool(name="pt_sb", bufs=10) as pt_pool, \
             tc.tile_pool(name="rb_sb", bufs=2) as rb_pool, \
             tc.tile_pool(name="zsb_sb", bufs=6) as zsb_pool, \
             tc.tile_pool(name="zs_sb", bufs=2) as zs_pool, \
             tc.tile_pool(name="out_sb", bufs=4) as out_pool, \
             tc.tile_pool(name="recip_dr", bufs=2, space="DRAM") as rdr_pool, \
             tc.tile_pool(name="recip_sb", bufs=2) as recip_pool:

            # ---------- projection building blocks (also used as fillers) --
            chain_defs = ((wq01, bq01, QT01), (wk01, bk01, KT01),
                          (wqk2, bqk2, QKT2))

            def emit_chain(n, ci):
                nsl = slice(n * 512, (n + 1) * 512)
                w_s, bias_s, out_s = chain_defs[ci]
                ps = proj_pool.tile([128, 512], F32, tag="chain",
                                    name=f"ch{n}_{ci}")
                for d in range(DCH):
                    nc.tensor.matmul(ps[:], w_s[:, d * 128:(d + 1) * 128],
                                     xt[d][:, nsl],
                                     start=(d == 0), stop=(d == DCH - 1))
                nc.vector.tensor_scalar_add(out_s[:, nsl], ps[:], bias_s[:])

            def emit_qk2x(n):
                nsl = slice(n * 512, (n + 1) * 512)
                nc.sync.dma_start(QK2x[0:64, nsl], QKT2[64:128, nsl])
                nc.sync.dma_start(QK2x[64:128, nsl], QKT2[0:64, nsl])

            def emit_vproj(s_t):
                ps = proj_pool.tile([128, 512], F32, tag="chain",
                                    name=f"v{s_t}")
                for d in range(DCH):
                    nc.tensor.matmul(ps[:, 0:192],
                                     xt[d][:, s_t * 128:(s_t + 1) * 128],
                                     wv[:, d * 192:(d + 1) * 192],
                                     start=(d == 0), stop=(d == DCH - 1))
                vdst = v_sb[:, s_t * 384:(s_t + 1) * 384] \
                    .rearrange("p (h c) -> p h c", h=3)[:, :, 0:64]
                nc.vector.tensor_add(
                    vdst, ps[:, 0:192].rearrange("p (h c) -> p h c", h=3),
                    bv[:].rearrange("p (h c) -> p h c", h=3))

            # head: just enough for q-block 0 (needs n=0 chains, V s-tiles
            # 0-3); everything else becomes fillers inside the rounds
            for ci in range(3):
                emit_chain(0, ci)
            emit_qk2x(0)
            for s_t in range(4):
                emit_vproj(s_t)

            fillers = []
            for n in range(1, 4):
                for ci in range(3):
                    fillers.append(lambda n=n, ci=ci: emit_chain(n, ci))
                fillers.append(lambda n=n: emit_qk2x(n))
                for s_t in range(4 * n, 4 * n + 4):
                    fillers.append(lambda s_t=s_t: emit_vproj(s_t))

            pending_tasks = []

            def make_op_emitter(qi):
                """Output projection for q-block qi, one 2-tile pair a call."""
                def emit(tp):
                    t0 = 4 * qi + 2 * tp
                    t1 = t0 + 1
                    sl0 = slice(t0 * 128, (t0 + 1) * 128)
                    sl1 = slice(t1 * 128, (t1 + 1) * 128)
                    ps0 = sT_pool.tile([128, D], F32, tag="sT", name=f"op{t0}")
                    ps1 = sT_pool.tile([128, D], F32, tag="sT", name=f"op{t1}")
                    for n0, nw in ((0, 512), (512, 256)):
                        nc.tensor.matmul(ps0[:, n0:n0 + nw], zT01[:, sl0],
                                         wo01[:, n0:n0 + nw],
                                         start=True, stop=False)
                        nc.tensor.matmul(ps1[:, n0:n0 + nw], zT01[:, sl1],
                                         wo01[:, n0:n0 + nw],
                                         start=True, stop=False)
                    for n0, nw in ((0, 512), (512, 256)):
                        # zT2 K=64: even tile rows 0-63, odd rows 64-127 ->
                        # row-group paired
                        nc.tensor.matmul(ps0[:, n0:n0 + nw], zT2x[0:64, sl0],
                                         wo2[0:64, n0:n0 + nw],
                                         start=False, stop=True)
                        nc.tensor.matmul(ps1[:, n0:n0 + nw],
                                         zT2x[64:128, sl1],
                                         wo2[64:128, n0:n0 + nw],
                                         start=False, stop=True)
                    for t, ps in ((t0, ps0), (t1, ps1)):
                        ob = out_pool.tile([128, D], F32, tag="ob")
                        if qi == NQB - 1:
                            # last block: ACT is past its exps, split halves
                            nc.vector.tensor_copy(ob[:, 0:384], ps[:, 0:384])
                            nc.scalar.copy(ob[:, 384:D], ps[:, 384:D])
                        else:
                            nc.vector.tensor_copy(ob[:], ps[:])
                        nc.sync.dma_start(out_d[t * 128:(t + 1) * 128, :], ob[:])
                return emit

            # ---------------- the attention stream ----------------
            gridx = 0          # global round counter (filler pacing)
            for qi in range(NQB):
                q0 = qi * QB
                J = 4 * qi + 4
                qsl = slice(q0, q0 + QB)

                def exp_mask(rr, st, name):
                    """exp+mask a [128, 1024] score tile whose halves are
                    k-tiles with diagonal offsets rr=(r0, r1); r<0 = fully
                    below diagonal. Masking runs on GpSimd."""
                    pt = pt_pool.tile([128, 1024], BF16, tag="pt", name=name)
                    s0 = rr[0] * 128 if rr[0] >= 0 else 0
                    nc.scalar.activation(pt[:, s0:1024], st[:, s0:1024], AF.Exp)
                    for jj, r in enumerate(rr):
                        off = jj * 512
                        if r >= 0:
                            if r > 0:
                                nc.gpsimd.memset(pt[:, off:off + r * 128], 0.0)
                            dsl = slice(off + r * 128, off + (r + 1) * 128)
                            nc.gpsimd.tensor_mul(pt[:, dsl], pt[:, dsl], tri[:])
                    return pt

                zts = [zT_pool.tile([128, 512], F32, tag="zT", name=f"zt{i}")
                       for i in range(3)]

                def pv_one(hv, j, pt, half):
                    nc.tensor.matmul(
                        zts[hv][:],
                        v_sb[:, j * 384 + hv * 128:j * 384 + (hv + 1) * 128],
                        pt[:, half * 512:(half + 1) * 512],
                        start=(j == 0), stop=(j == J - 1))

                # PV queues: h0 skew 2, h1/h2 skew 4 — matches when the
                # previous block's zsb copies release the 3 zts slots
                q_fast = []   # (ridx, hv, j, pt, half)
                q_slow = []

                rounds = [("p", j) for j in range(J)] + \
                         [("2", jp) for jp in range(J // 2)]
                n_r = len(rounds)
                # tasks deferred from the previous block (normalize parts,
                # out-proj), due at rounds where their DMA deps have landed
                # so they never block an engine queue
                op_sched = [[min(due, n_r - 1), fn]
                            for due, fn in pending_tasks]
                pending_tasks = []

                for ridx, (kind, idx) in enumerate(rounds):
                    new_sched = []
                    for ent in op_sched:
                        if ent[0] <= ridx:
                            ent[1]()
                        else:
                            new_sched.append(ent)
                    op_sched = new_sched

                    st = sT_pool.tile([128, 1024], F32, tag="sT",
                                      name=f"st_{kind}{idx}")
                    if kind == "p":
                        j = idx
                        ksl = slice(j * 128, (j + 1) * 128)
                        nc.tensor.matmul(st[:, 0:512], KT01[0:64, ksl],
                                         QT01[0:64, qsl], start=True, stop=True)
                        nc.tensor.matmul(st[:, 512:1024], KT01[64:128, ksl],
                                         QT01[64:128, qsl], start=True, stop=True)
                        rr = (idx - 4 * qi, idx - 4 * qi)
                    else:
                        j0, j1 = 2 * idx, 2 * idx + 1
                        nc.tensor.matmul(st[:, 0:512],
                                         QK2x[0:64, j0 * 128:(j0 + 1) * 128],
                                         QKT2[0:64, qsl], start=True, stop=True)
                        nc.tensor.matmul(st[:, 512:1024],
                                         QKT2[64:128, j1 * 128:(j1 + 1) * 128],
                                         QK2x[64:128, qsl], start=True, stop=True)
                        rr = (j0 - 4 * qi, j1 - 4 * qi)

                    # projection fillers: every round while q-block 0 has PE
                    # slack, then every other round (always ahead of the PVs
                    # that consume their outputs)
                    if fillers and (gridx < 8 or gridx % 2 == 0):
                        fillers.pop(0)()
                    gridx += 1

                    while q_fast and q_fast[0][0] <= ridx - 2:
                        pv_one(*q_fast.pop(0)[1:])
                    while q_slow and q_slow[0][0] <= ridx - 4:
                        pv_one(*q_slow.pop(0)[1:])

                    pt = exp_mask(rr, st, f"pt{kind}{idx}")
                    if kind == "p":
                        q_fast.append((ridx, 0, idx, pt, 0))
                        q_slow.append((ridx, 1, idx, pt, 1))
                    else:
                        q_slow.append((ridx, 2, 2 * idx, pt, 0))
                        q_slow.append((ridx, 2, 2 * idx + 1, pt, 1))

                for task in q_fast:
                    pv_one(*task[1:])
                for task in q_slow:
                    pv_one(*task[1:])
                for ent in op_sched:
                    ent[1]()

                # ---- free the zts PSUM slots NOW: single copy to SBUF,
                # then launch the denominator bounce (DMA-only) ----
                zsbs = []
                for h in range(3):
                    zsb = zsb_pool.tile([65, 512], F32, tag="zsb",
                                        name=f"zsb{h}")
                    nc.vector.tensor_copy(zsb[:], zts[h][0:65, :])
                    zsbs.append(zsb)
                dr1 = rdr_pool.tile([1, 3 * 512], F32, tag="dr1")
                for h in range(3):
                    nc.sync.dma_start(dr1[0:1, h * 512:(h + 1) * 512],
                                      zsbs[h][64:65, :])
                rs = recip_pool.tile([128, 12], F32, tag="rs")
                nc.sync.dma_start(
                    rs[:], dr1[:].rearrange("o (p f) -> (o p) f", p=128))

                # the DVE/DMA parts below run as deferred tasks in the next
                # block, after their DMA deps have landed
                def norm_b1(rs=rs, qsl=qsl):
                    rr_t = recip_pool.tile([128, 12], F32, tag="rr")
                    nc.vector.reciprocal(rr_t[:], rs[:])
                    dr2 = rdr_pool.tile([1, 3 * 512], F32, tag="dr2")
                    nc.sync.dma_start(
                        dr2[:].rearrange("o (p f) -> (o p) f", p=128), rr_t[:])
                    rb = rb_pool.tile([64, 3 * 512], F32, tag="rb")
                    for h in range(3):
                        nc.sync.dma_start(
                            rb[:, h * 512:(h + 1) * 512],
                            dr2[0:1, h * 512:(h + 1) * 512]
                            .broadcast_to([64, 512]))
                    return rb

                rb_box = []

                def norm_b1_run(rb_box=rb_box, rs=rs, qsl=qsl):
                    rb_box.append(norm_b1(rs, qsl))

                def norm_b2(rb_box=rb_box, zsbs=zsbs, qsl=qsl):
                    rb = rb_box[0]
                    nc.vector.tensor_mul(zT01[0:64, qsl], zsbs[0][0:64, :],
                                         rb[:, 0:512])
                    z1 = zs_pool.tile([64, 512], BF16, tag="z1")
                    nc.vector.tensor_mul(z1[:], zsbs[1][0:64, :],
                                         rb[:, 512:1024])
                    nc.sync.dma_start(zT01[64:128, qsl], z1[:])
                    z2 = zs_pool.tile([64, 512], BF16, tag="z2")
                    nc.vector.tensor_mul(z2[:], zsbs[2][0:64, :],
                                         rb[:, 1024:1536])
                    nc.sync.dma_start(zT2x[0:64, qsl], z2[:])
                    nc.sync.dma_start(zT2x[64:128, qsl], z2[:])

                op_emit = make_op_emitter(qi)
                pending_tasks = [(5, norm_b1_run), (9, norm_b2),
                                 (12, lambda op_emit=op_emit: op_emit(0)),
                                 (15, lambda op_emit=op_emit: op_emit(1))]

            # last q-block: flush its normalize + output projection (tail)
            for due, fn in pending_tasks:
                fn()

    nc.compile()
    return nc


def _get_nc():
    global _BUILT
    if _BUILT is None:
        _BUILT = build_nc()
    return _BUILT


def make_in_maps(inputs):
    x = np.asarray(inputs["normalized_resid_pre"], dtype=np.float32)
    W_Q = np.asarray(inputs["W_Q"], dtype=np.float32)
    W_K = np.asarray(inputs["W_K"], dtype=np.float32)
    W_V = np.asarray(inputs["W_V"], dtype=np.float32)
    W_O = np.asarray(inputs["W_O"], dtype=np.float32)
    b_Q = np.asarray(inputs["b_Q"], dtype=np.float32)
    b_K = np.asarray(inputs["b_K"], dtype=np.float32)
    b_V = np.asarray(inputs["b_V"], dtype=np.float32)
    sc = 1.0 / np.sqrt(np.float32(DH))

    in_maps = []
    for c in range(N_CORES):
        b = c // 4
        h = (c % 4) * 3
        hs = [h, h + 1, h + 2]
        m = {
            "xT": np.ascontiguousarray(x[b].T).astype(NPBF16),
            "wq01": np.concatenate([W_Q[hs[0]] * sc, W_Q[hs[1]] * sc],
                                   axis=1).astype(NPBF16),
            "wk01": np.concatenate([W_K[hs[0]], W_K[hs[1]]], axis=1).astype(NPBF16),
            "wqk2": np.concatenate([W_Q[hs[2]] * sc, W_K[hs[2]]],
                                   axis=1).astype(NPBF16),
            "wv": np.concatenate([W_V[hh] for hh in hs],
                                 axis=1).astype(NPBF16),
            "wo01": np.concatenate([W_O[hs[0]], W_O[hs[1]]], axis=0).astype(NPBF16),
            "wo2": np.concatenate([W_O[hs[2]], W_O[hs[2]]], axis=0).astype(NPBF16),
            "bq01": (np.concatenate([b_Q[hs[0]], b_Q[hs[1]]]) * sc)[:, None]
                    .astype(np.float32),
            "bk01": np.concatenate([b_K[hs[0]], b_K[hs[1]]])[:, None]
                    .astype(np.float32),
            "bqk2": np.concatenate([b_Q[hs[2]] * sc, b_K[hs[2]]])[:, None]
                    .astype(np.float32),
            "bv": np.ascontiguousarray(np.broadcast_to(
                np.concatenate([b_V[hh] for hh in hs]),
                (128, 192))).astype(np.float32),
        }
        in_maps.append(m)
    return in_maps


def kernel(**inputs):
    global LAST_EXEC_NS, LAST_RESULT
    nc = _get_nc()
    in_maps = make_in_maps(inputs)
    b_O = np.asarray(inputs["b_O"], dtype=np.float32)

    res = run_bass_kernel_spmd(nc, in_maps, core_ids=list(range(N_CORES)),
                               trace=TRACE_ENABLED)
    LAST_EXEC_NS = res.exec_time_ns
    LAST_RESULT = res
    parts = [r["out_p"] for r in res.results]
    out0 = parts[0] + parts[1] + parts[2] + parts[3]
    out1 = parts[4] + parts[5] + parts[6] + parts[7]
    out = np.stack([out0, out1]) + b_O
    return out.astype(np.float32)


# revision 39
# speedup vs baseline: 1.0352x; 1.0352x over previous
"""Multi-head causal attention (dense transformer block) on 8 TRN2 NeuronCores.

Problem: B=2, S=2048, D_MODEL=768, H=12 heads, D_HEAD=64, fp32 I/O.

Sharding: 24 (batch, head) units over 8 cores -> 3 heads x 1 batch per core.
Cores 0-3 handle batch 0 (heads 0-2, 3-5, 6-8, 9-11), cores 4-7 batch 1.
Each core computes its heads' contribution to out[b] = sum_h z_h @ W_O[h];
the host sums the 4 partials per batch and adds b_O.

The kernel is one software-pipelined stream built around the ScalarE exp
cadence and a paced deferred-work FIFO:

  - scores TRANSPOSED: sT[k, q] = KT^T-slice x QT (K=64). Each PSUM score
    tile [128, 1024] holds both halves of a row-group pair ([h0_j | h1_j]
    or [h2_j | h2_j+1]); two tiles' score matmuls are emitted back-to-back
    (4 alternating-half K=64 matmuls) which the PE runs at ~2x via
    row-group concurrency (isolated pairs only reach ~1.5x).
  - All other PE/DVE work (PV matmuls, PSUM->SBUF z copies, denominator
    bounce DMAs, Q/K/V projection chains for later blocks) flows through
    one FIFO pumped a few tasks per round: order guarantees correctness
    (writers precede readers), pacing keeps any engine queue from ever
    blocking more than ~1 round ahead of the exp stream.
  - PV: zT_h += [V_h | ones64]^T @ PT (M=128, FWL-eligible); PSUM slot is
    released right after a single [65,512] copy to SBUF; the softmax
    normalize (1/den via a [128,12] DRAM-bounce + muls) runs from SBUF as
    due-scheduled tasks in the NEXT block, entirely off the critical path.
  - causal masking (memset + triangular multiply) on the GpSimd engine.
  - output projection of block n injected mid-block n+1; zT2's K=64
    matmuls of adjacent q-tiles row-group paired via a zT2 high-half copy.
"""

import numpy as np
import ml_dtypes
from contextlib import ExitStack

import concourse.bass as bass
import concourse.mybir as mybir
import concourse.tile as tile
from concourse import bacc
from concourse.bass_utils import run_bass_kernel_spmd

BF16 = mybir.dt.bfloat16
F32 = mybir.dt.float32
AF = mybir.ActivationFunctionType
NPBF16 = ml_dtypes.bfloat16

B, S, D, H, DH = 2, 2048, 768, 12, 64
N_CORES = 8
DCH = D // 128
NKT = S // 128
QB = 512
NQB = S // QB

TRACE_ENABLED = False
LAST_EXEC_NS = None
LAST_RESULT = None
_BUILT = None


def build_nc():
    nc = bacc.Bacc("TRN2", target_bir_lowering=False, debug=False)

    xT_d = nc.dram_tensor("xT", [D, S], BF16, kind="ExternalInput")
    wq01_d = nc.dram_tensor("wq01", [D, 128], BF16, kind="ExternalInput")
    wk01_d = nc.dram_tensor("wk01", [D, 128], BF16, kind="ExternalInput")
    wqk2_d = nc.dram_tensor("wqk2", [D, 128], BF16, kind="ExternalInput")
    wv_d = nc.dram_tensor("wv", [D, 192], BF16, kind="ExternalInput")
    wo01_d = nc.dram_tensor("wo01", [128, D], BF16, kind="ExternalInput")
    wo2_d = nc.dram_tensor("wo2", [128, D], BF16, kind="ExternalInput")
    bq01_d = nc.dram_tensor("bq01", [128, 1], F32, kind="ExternalInput")
    bk01_d = nc.dram_tensor("bk01", [128, 1], F32, kind="ExternalInput")
    bqk2_d = nc.dram_tensor("bqk2", [128, 1], F32, kind="ExternalInput")
    bv_d = nc.dram_tensor("bv", [128, 192], F32, kind="ExternalInput")
    out_d = nc.dram_tensor("out_p", [S, D], F32, kind="ExternalOutput")

    tri_np = np.triu(np.ones((128, 128), np.float32)).astype(NPBF16)
    tri_d = nc.inline_tensor(tri_np, "tri")

    with tile.TileContext(nc) as tc, ExitStack() as ctx:
        persist = ctx.enter_context(tc.tile_pool(name="persist", bufs=1))

        # ---- HAM warm-up across the input-DMA wait ----
        with tc.tile_pool(name="warm_ps", bufs=1, space="PSUM") as warm_pool:
            wz = persist.tile([128, 128], BF16, tag="wz")
            nc.vector.memset(wz[:], 0.0)
            wps = warm_pool.tile([128, 128], F32, tag="warm")
            for _ in range(24):
                nc.tensor.matmul(wps[:], wz[:], wz[:], start=True, stop=True)

        # ---- stage inputs: xT chunks first (they gate the projections),
        # issue split across two engines to halve issue serialization;
        # W_O last (needed ~60us in) ----
        def load_w(dram, cols, tag):
            t = persist.tile([128, DCH * cols], BF16, tag=tag)
            a = dram[:, :]
            src = bass.AP(tensor=a.tensor, offset=a.offset,
                          ap=[[cols, 128], [128 * cols, DCH], [1, cols]])
            nc.sync.dma_start(t[:].rearrange("p (c f) -> p c f", c=DCH), src)
            return t

        xt = [persist.tile([128, S], BF16, tag=f"xt{d}", name=f"xt{d}")
              for d in range(DCH)]
        nc.sync.dma_start(xt[0][:], xT_d[0:128, :])
        nc.gpsimd.dma_start(xt[1][:], xT_d[128:256, :])
        wq01 = load_w(wq01_d, 128, "wq01")
        nc.gpsimd.dma_start(xt[2][:], xT_d[256:384, :])
        wk01 = load_w(wk01_d, 128, "wk01")
        nc.gpsimd.dma_start(xt[3][:], xT_d[384:512, :])
        wqk2 = load_w(wqk2_d, 128, "wqk2")
        nc.gpsimd.dma_start(xt[4][:], xT_d[512:640, :])
        wv = load_w(wv_d, 192, "wv")
        nc.gpsimd.dma_start(xt[5][:], xT_d[640:768, :])

        def load_small(dram, shape, dt, tag):
            t = persist.tile(shape, dt, tag=tag)
            nc.sync.dma_start(t[:], dram[:, :])
            return t

        bq01 = load_small(bq01_d, [128, 1], F32, "bq01")
        bk01 = load_small(bk01_d, [128, 1], F32, "bk01")
        bqk2 = load_small(bqk2_d, [128, 1], F32, "bqk2")
        bv = load_small(bv_d, [128, 192], F32, "bv")
        tri = load_small(tri_d, [128, 128], BF16, "tri")

        wo01 = persist.tile([128, D], BF16, tag="wo01")
        nc.sync.dma_start(wo01[:], wo01_d[:, :])
        wo2 = persist.tile([128, D], BF16, tag="wo2")
        nc.sync.dma_start(wo2[:], wo2_d[:, :])

        # ---- persistent intermediates ----
        QT01 = persist.tile([128, S], BF16, tag="QT01")
        KT01 = persist.tile([128, S], BF16, tag="KT01")
        QKT2 = persist.tile([128, S], BF16, tag="QKT2")
        QK2x = persist.tile([128, S], BF16, tag="QK2x")
        v_sb = persist.tile([128, NKT * 384], BF16, tag="v_sb")
        nc.vector.memset(
            v_sb[:].rearrange("p (t h c) -> p t h c", t=NKT, h=3)[:, :, :, 64:128],
            1.0)
        zT01 = persist.tile([128, S], BF16, tag="zT01")
        zT2x = persist.tile([128, S], BF16, tag="zT2x")

        with tc.tile_pool(name="proj_ps", bufs=1, space="PSUM") as proj_pool, \
             tc.tile_pool(name="sT_ps", bufs=2, space="PSUM") as sT_pool, \
             tc.tile_pool(name="zT_ps", bufs=3, space="PSUM") as zT_pool, \
             tc.tile_pool(name="pt_sb", bufs=12) as pt_pool, \
             tc.tile_pool(name="rb_sb", bufs=2) as rb_pool, \
             tc.tile_pool(name="zsb_sb", bufs=6) as zsb_pool, \
             tc.tile_pool(name="zs_sb", bufs=2) as zs_pool, \
             tc.tile_pool(name="out_sb", bufs=4) as out_pool, \
             tc.tile_pool(name="recip_dr", bufs=2, space="DRAM") as rdr_pool, \
             tc.tile_pool(name="recip_sb", bufs=2) as recip_pool:

            chain_defs = ((wq01, bq01, QT01), (wk01, bk01, KT01),
                          (wqk2, bqk2, QKT2))

            def emit_chain(n, ci):
                nsl = slice(n * 512, (n + 1) * 512)
                w_s, bias_s, out_s = chain_defs[ci]
                ps = proj_pool.tile([128, 512], F32, tag="chain",
                                    name=f"ch{n}_{ci}")
                for d in range(DCH):
                    nc.tensor.matmul(ps[:], w_s[:, d * 128:(d + 1) * 128],
                                     xt[d][:, nsl],
                                     start=(d == 0), stop=(d == DCH - 1))
                nc.vector.tensor_scalar_add(out_s[:, nsl], ps[:], bias_s[:])

            def emit_qk2x(n):
                nsl = slice(n * 512, (n + 1) * 512)
                nc.sync.dma_start(QK2x[0:64, nsl], QKT2[64:128, nsl])
                nc.sync.dma_start(QK2x[64:128, nsl], QKT2[0:64, nsl])

            def emit_vproj(s_t):
                ps = proj_pool.tile([128, 512], F32, tag="chain",
                                    name=f"v{s_t}")
                for d in range(DCH):
                    nc.tensor.matmul(ps[:, 0:192],
                                     xt[d][:, s_t * 128:(s_t + 1) * 128],
                                     wv[:, d * 192:(d + 1) * 192],
                                     start=(d == 0), stop=(d == DCH - 1))
                vdst = v_sb[:, s_t * 384:(s_t + 1) * 384] \
                    .rearrange("p (h c) -> p h c", h=3)[:, :, 0:64]
                nc.vector.tensor_add(
                    vdst, ps[:, 0:192].rearrange("p (h c) -> p h c", h=3),
                    bv[:].rearrange("p (h c) -> p h c", h=3))

            # head: the projections q-block 0 needs right away
            for ci in range(3):
                emit_chain(0, ci)
            emit_qk2x(0)
            for s_t in range(4):
                emit_vproj(s_t)

            # deferred-work FIFO: projection fillers first, then PVs etc.
            # get appended as the stream runs
            work = []
            for n in range(1, 4):
                for ci in range(3):
                    work.append(lambda n=n, ci=ci: emit_chain(n, ci))
                work.append(lambda n=n: emit_qk2x(n))
                for s_t in range(4 * n, 4 * n + 4):
                    work.append(lambda s_t=s_t: emit_vproj(s_t))

            def pump(n):
                for _ in range(n):
                    if work:
                        work.pop(0)()

            def make_op_emitter(qi):
                def emit(tp):
                    t0 = 4 * qi + 2 * tp
                    t1 = t0 + 1
                    sl0 = slice(t0 * 128, (t0 + 1) * 128)
                    sl1 = slice(t1 * 128, (t1 + 1) * 128)
                    ps0 = sT_pool.tile([128, D], F32, tag="sT", name=f"op{t0}")
                    ps1 = sT_pool.tile([128, D], F32, tag="sT", name=f"op{t1}")
                    for n0, nw in ((0, 512), (512, 256)):
                        nc.tensor.matmul(ps0[:, n0:n0 + nw], zT01[:, sl0],
                                         wo01[:, n0:n0 + nw],
                                         start=True, stop=False)
                        nc.tensor.matmul(ps1[:, n0:n0 + nw], zT01[:, sl1],
                                         wo01[:, n0:n0 + nw],
                                         start=True, stop=False)
                    for n0, nw in ((0, 512), (512, 256)):
                        nc.tensor.matmul(ps0[:, n0:n0 + nw], zT2x[0:64, sl0],
                                         wo2[0:64, n0:n0 + nw],
                                         start=False, stop=True)
                        nc.tensor.matmul(ps1[:, n0:n0 + nw],
                                         zT2x[64:128, sl1],
                                         wo2[64:128, n0:n0 + nw],
                                         start=False, stop=True)
                    for t, ps in ((t0, ps0), (t1, ps1)):
                        ob = out_pool.tile([128, D], F32, tag="ob")
                        if qi == NQB - 1:
                            nc.vector.tensor_copy(ob[:, 0:384], ps[:, 0:384])
                            nc.scalar.copy(ob[:, 384:D], ps[:, 384:D])
                        else:
                            nc.vector.tensor_copy(ob[:], ps[:])
                        nc.sync.dma_start(out_d[t * 128:(t + 1) * 128, :], ob[:])
                return emit

            pending = []      # [due_ridx, fn] carried into the next q-block

            for qi in range(NQB):
                q0 = qi * QB
                J = 4 * qi + 4
                qsl = slice(q0, q0 + QB)

                def exp_mask(rr, st, name):
                    pt = pt_pool.tile([128, 1024], BF16, tag="pt", name=name)
                    s0 = rr[0] * 128 if rr[0] >= 0 else 0
                    nc.scalar.activation(pt[:, s0:1024], st[:, s0:1024], AF.Exp)
                    for jj, r in enumerate(rr):
                        off = jj * 512
                        if r >= 0:
                            if r > 0:
                                nc.gpsimd.memset(pt[:, off:off + r * 128], 0.0)
                            dsl = slice(off + r * 128, off + (r + 1) * 128)
                            nc.gpsimd.tensor_mul(pt[:, dsl], pt[:, dsl], tri[:])
                    return pt

                zts = [zT_pool.tile([128, 512], F32, tag="zT", name=f"zt{i}")
                       for i in range(3)]

                def pv_one(hv, j, pt, half, zts=zts, J=J):
                    nc.tensor.matmul(
                        zts[hv][:],
                        v_sb[:, j * 384 + hv * 128:j * 384 + (hv + 1) * 128],
                        pt[:, half * 512:(half + 1) * 512],
                        start=(j == 0), stop=(j == J - 1))

                def emit_sc(kind, idx):
                    st = sT_pool.tile([128, 1024], F32, tag="sT",
                                      name=f"st_{kind}{idx}_{qi}")
                    if kind == "p":
                        j = idx
                        ksl = slice(j * 128, (j + 1) * 128)
                        nc.tensor.matmul(st[:, 0:512], KT01[0:64, ksl],
                                         QT01[0:64, qsl], start=True, stop=True)
                        nc.tensor.matmul(st[:, 512:1024], KT01[64:128, ksl],
                                         QT01[64:128, qsl], start=True, stop=True)
                        rr = (idx - 4 * qi, idx - 4 * qi)
                    else:
                        j0, j1 = 2 * idx, 2 * idx + 1
                        nc.tensor.matmul(st[:, 0:512],
                                         QK2x[0:64, j0 * 128:(j0 + 1) * 128],
                                         QKT2[0:64, qsl], start=True, stop=True)
                        nc.tensor.matmul(st[:, 512:1024],
                                         QKT2[64:128, j1 * 128:(j1 + 1) * 128],
                                         QK2x[64:128, qsl], start=True, stop=True)
                        rr = (j0 - 4 * qi, j1 - 4 * qi)
                    return st, rr

                rounds = [("p", j) for j in range(J)] + \
                         [("2", jp) for jp in range(J // 2)]
                n_r = len(rounds)
                sched = [[min(due, n_r - 1), fn] for due, fn in pending]
                pending = []

                # process tiles two at a time: the 4 K=64 score matmuls
                # cluster and keep the 2x row-group rate
                for step in range(n_r // 2):
                    r0, r1 = 2 * step, 2 * step + 1
                    still = []
                    for ent in sched:
                        if ent[0] <= r1:
                            ent[1]()
                        else:
                            still.append(ent)
                    sched = still

                    ka, ia = rounds[r0]
                    kb, ib = rounds[r1]
                    st_a, rr_a = emit_sc(ka, ia)
                    st_b, rr_b = emit_sc(kb, ib)

                    pump(8 if qi == 0 else 5)

                    pt_a = exp_mask(rr_a, st_a, f"pt{ka}{ia}_{qi}")
                    pt_b = exp_mask(rr_b, st_b, f"pt{kb}{ib}_{qi}")

                    for kind, idx, pt in ((ka, ia, pt_a), (kb, ib, pt_b)):
                        if kind == "p":
                            work.append(lambda i=idx, p=pt, f=pv_one: f(0, i, p, 0))
                            work.append(lambda i=idx, p=pt, f=pv_one: f(1, i, p, 1))
                        else:
                            work.append(
                                lambda i=idx, p=pt, f=pv_one: f(2, 2 * i, p, 0))
                            work.append(
                                lambda i=idx, p=pt, f=pv_one: f(2, 2 * i + 1, p, 1))

                # run any not-yet-due carried tasks now (end of block)
                for ent in sched:
                    ent[1]()

                # ---- push z-copy + denominator-bounce tasks; the PV tasks
                # for this block are still ahead of them in the FIFO ----
                zsbs = [zsb_pool.tile([65, 512], F32, tag="zsb",
                                      name=f"zsb{h}_{qi}") for h in range(3)]
                dr1 = rdr_pool.tile([1, 3 * 512], F32, tag="dr1")
                rs = recip_pool.tile([128, 12], F32, tag="rs")

                def zcopy(h, zsbs=zsbs, zts=zts):
                    nc.vector.tensor_copy(zsbs[h][:], zts[h][0:65, :])

                bounced = [False]

                def bounce1(zsbs=zsbs, dr1=dr1, rs=rs, bounced=bounced):
                    for h in range(3):
                        nc.sync.dma_start(dr1[0:1, h * 512:(h + 1) * 512],
                                          zsbs[h][64:65, :])
                    nc.sync.dma_start(
                        rs[:], dr1[:].rearrange("o (p f) -> (o p) f", p=128))
                    bounced[0] = True

                for h in range(3):
                    work.append(lambda h=h: zcopy(h))
                work.append(bounce1)

                rb_box = []

                def norm_b1(rs=rs, rb_box=rb_box, bounced=bounced):
                    while not bounced[0]:
                        pump(1)
                    rr_t = recip_pool.tile([128, 12], F32, tag="rr")
                    nc.vector.reciprocal(rr_t[:], rs[:])
                    dr2 = rdr_pool.tile([1, 3 * 512], F32, tag="dr2")
                    nc.sync.dma_start(
                        dr2[:].rearrange("o (p f) -> (o p) f", p=128), rr_t[:])
                    rb = rb_pool.tile([64, 3 * 512], F32, tag="rb")
                    for h in range(3):
                        nc.sync.dma_start(
                            rb[:, h * 512:(h + 1) * 512],
                            dr2[0:1, h * 512:(h + 1) * 512]
                            .broadcast_to([64, 512]))
                    rb_box.append(rb)

                def norm_b2(rb_box=rb_box, zsbs=zsbs, qsl=qsl):
                    rb = rb_box[0]
                    nc.vector.tensor_mul(zT01[0:64, qsl], zsbs[0][0:64, :],
                                         rb[:, 0:512])
                    z1 = zs_pool.tile([64, 512], BF16, tag="z1")
                    nc.vector.tensor_mul(z1[:], zsbs[1][0:64, :],
                                         rb[:, 512:1024])
                    nc.sync.dma_start(zT01[64:128, qsl], z1[:])
                    z2 = zs_pool.tile([64, 512], BF16, tag="z2")
                    nc.vector.tensor_mul(z2[:], zsbs[2][0:64, :],
                                         rb[:, 1024:1536])
                    nc.sync.dma_start(zT2x[0:64, qsl], z2[:])
                    nc.sync.dma_start(zT2x[64:128, qsl], z2[:])

                op_emit = make_op_emitter(qi)
                pending = [(6, norm_b1), (10, norm_b2),
                           (13, lambda op_emit=op_emit: op_emit(0)),
                           (16, lambda op_emit=op_emit: op_emit(1))]

            # ---- tail: drain everything for the last q-block ----
            pump(len(work))
            for due, fn in pending:
                fn()

    nc.compile()
    return nc


def _get_nc():
    global _BUILT
    if _BUILT is None:
        _BUILT = build_nc()
    return _BUILT


def make_in_maps(inputs):
    x = np.asarray(inputs["normalized_resid_pre"], dtype=np.float32)
    W_Q = np.asarray(inputs["W_Q"], dtype=np.float32)
    W_K = np.asarray(inputs["W_K"], dtype=np.float32)
    W_V = np.asarray(inputs["W_V"], dtype=np.float32)
    W_O = np.asarray(inputs["W_O"], dtype=np.float32)
    b_Q = np.asarray(inputs["b_Q"], dtype=np.float32)
    b_K = np.asarray(inputs["b_K"], dtype=np.float32)
    b_V = np.asarray(inputs["b_V"], dtype=np.float32)
    sc = 1.0 / np.sqrt(np.float32(DH))

    in_maps = []
    for c in range(N_CORES):
        b = c // 4
        h = (c % 4) * 3
        hs = [h, h + 1, h + 2]
        m = {
            "xT": np.ascontiguousarray(x[b].T).astype(NPBF16),
            "wq01": np.concatenate([W_Q[hs[0]] * sc, W_Q[hs[1]] * sc],
                                   axis=1).astype(NPBF16),
            "wk01": np.concatenate([W_K[hs[0]], W_K[hs[1]]], axis=1).astype(NPBF16),
            "wqk2": np.concatenate([W_Q[hs[2]] * sc, W_K[hs[2]]],
                                   axis=1).astype(NPBF16),
            "wv": np.concatenate([W_V[hh] for hh in hs],
                                 axis=1).astype(NPBF16),
            "wo01": np.concatenate([W_O[hs[0]], W_O[hs[1]]], axis=0).astype(NPBF16),
            "wo2": np.concatenate([W_O[hs[2]], W_O[hs[2]]], axis=0).astype(NPBF16),
            "bq01": (np.concatenate([b_Q[hs[0]], b_Q[hs[1]]]) * sc)[:, None]
                    .astype(np.float32),
            "bk01": np.concatenate([b_K[hs[0]], b_K[hs[1]]])[:, None]
                    .astype(np.float32),
            "bqk2": np.concatenate([b_Q[hs[2]] * sc, b_K[hs[2]]])[:, None]
                    .astype(np.float32),
            "bv": np.ascontiguousarray(np.broadcast_to(
                np.concatenate([b_V[hh] for hh in hs]),
                (128, 192))).astype(np.float32),
        }
        in_maps.append(m)
    return in_maps


def kernel(**inputs):
    global LAST_EXEC_NS, LAST_RESULT
    nc = _get_nc()
    in_maps = make_in_maps(inputs)
    b_O = np.asarray(inputs["b_O"], dtype=np.float32)

    res = run_bass_kernel_spmd(nc, in_maps, core_ids=list(range(N_CORES)),
                               trace=TRACE_ENABLED)
    LAST_EXEC_NS = res.exec_time_ns
    LAST_RESULT = res
    parts = [r["out_p"] for r in res.results]
    out0 = parts[0] + parts[1] + parts[2] + parts[3]
    out1 = parts[4] + parts[5] + parts[6] + parts[7]
    out = np.stack([out0, out1]) + b_O
    return out.astype(np.float32)
